# revision 1
# baseline (speedup 1.0000x reference)
"""Masked fractional Hamming distance over 31 circular rotations, on 8 trn2 cores.

Math: for shift s, num(s)/den(s) with
  den(s) = sum maskbits = corr(ma, mb)(2s)        (l,k fused -> lag 2s)
  num(s) = masked differing bits; with the sign-encode
  A = (ia<<7)|ma, B = (ib<<7)|mb read as fp8e4m3 the bytes become
  {+0, -0, +2^-9, -2^-9} (sign=iris, magnitude=mask), so
  corr(A, B)(2s) = (den - 2*num) * 2^-18, corr(ma, mb raw bytes) = den * 2^-18.
Both correlations are computed as banded matmuls on the PE: contraction over
rows (128/partition group), stationary = 128-column chunk of the A side,
moving = 188-column window of the (30-halo-padded) B side; every chunk and
row-group accumulates into one (128,188) PSUM tile per pair since the
diagonal offset d = j - i - 30 is tiling-invariant. Band diagonals are summed
on the host (exact integers scaled by 2^-18).
"""

import numpy as np

N_CORES = 8
B_FULL, L = 4096, 2048
R = 15
J = 2 * L                      # fused (l, k) axis, circular shifts = even lags
B_SH = B_FULL // N_CORES       # 512 batches per core
ROWS = 2 * B_SH                # 1024 rows of length J per core
HALO = 2 * R                   # 30
NW = 128 + 2 * HALO            # 188 moving window
N_GROUPS = ROWS // 128         # 8
N_CHUNKS = J // 128            # 32

_CACHE = {}


def _build_program():
    import concourse.bass as bass
    import concourse.tile as tile
    from concourse import bacc, mybir

    u8 = mybir.dt.uint8
    u16 = mybir.dt.uint16
    f8 = mybir.dt.float8e4
    f32 = mybir.dt.float32
    Alu = mybir.AluOpType

    nc = bass.Bass()
    ia_d = nc.declare_dram_parameter("ia", [ROWS, J], u8, isOutput=False)
    ma_d = nc.declare_dram_parameter("ma", [ROWS, J], u8, isOutput=False)
    ib_d = nc.declare_dram_parameter("ib", [ROWS, J], u8, isOutput=False)
    mb_d = nc.declare_dram_parameter("mb", [ROWS, J], u8, isOutput=False)
    out_d = nc.declare_dram_parameter("out", [2, 128, NW], f32, isOutput=True)

    with tile.TileContext(nc) as tc:
        with (
            tc.tile_pool(name="raw", bufs=3) as raw_pool,
            tc.tile_pool(name="enc", bufs=3) as enc_pool,
            tc.tile_pool(name="acc", bufs=1, space="PSUM") as psum_pool,
        ):
            ps_ab = psum_pool.tile([128, NW], f32)
            ps_mm = psum_pool.tile([128, NW], f32)

            for g in range(N_GROUPS):
                rows = slice(g * 128, (g + 1) * 128)
                ia_t = raw_pool.tile([128, J], u8, tag="ia")
                ma_t = raw_pool.tile([128, J], u8, tag="ma")
                ib_t = raw_pool.tile([128, J], u8, tag="ib")
                mb_t = raw_pool.tile([128, J + 2 * HALO], u8, tag="mb")
                a_t = enc_pool.tile([128, J], u8, tag="A")
                b_t = enc_pool.tile([128, J + 2 * HALO], u8, tag="B")

                nc.sync.dma_start(ia_t[:], ia_d[rows, :])
                nc.sync.dma_start(ma_t[:], ma_d[rows, :])
                nc.sync.dma_start(ib_t[:], ib_d[rows, :])
                nc.sync.dma_start(mb_t[:, HALO : HALO + J], mb_d[rows, :])

                # A = (ia << 7) | ma per byte, done on u16-viewed data (both
                # bytes of a pair are {0,1}: the shift never crosses bytes).
                nc.vector.tensor_scalar_mul(
                    a_t[:].bitcast(u16), ia_t[:].bitcast(u16), 128.0
                )
                nc.vector.tensor_tensor(
                    a_t[:].bitcast(u16),
                    a_t[:].bitcast(u16),
                    ma_t[:].bitcast(u16),
                    op=Alu.bitwise_or,
                )
                nc.vector.tensor_scalar_mul(
                    b_t[:, HALO : HALO + J].bitcast(u16),
                    ib_t[:].bitcast(u16),
                    128.0,
                )
                nc.vector.tensor_tensor(
                    b_t[:, HALO : HALO + J].bitcast(u16),
                    b_t[:, HALO : HALO + J].bitcast(u16),
                    mb_t[:, HALO : HALO + J].bitcast(u16),
                    op=Alu.bitwise_or,
                )
                # circular halos for the moving-side tiles
                nc.vector.tensor_copy(b_t[:, 0:HALO], b_t[:, J : J + HALO])
                nc.vector.tensor_copy(b_t[:, HALO + J :], b_t[:, HALO : 2 * HALO])
                nc.vector.tensor_copy(mb_t[:, 0:HALO], mb_t[:, J : J + HALO])
                nc.vector.tensor_copy(mb_t[:, HALO + J :], mb_t[:, HALO : 2 * HALO])

                for c in range(N_CHUNKS):
                    a0 = c * 128
                    first = g == 0 and c == 0
                    last = g == N_GROUPS - 1 and c == N_CHUNKS - 1
                    nc.tensor.matmul(
                        ps_ab[:],
                        a_t[:, a0 : a0 + 128].bitcast(f8),
                        b_t[:, a0 : a0 + NW].bitcast(f8),
                        start=first,
                        stop=last,
                    )
                    nc.tensor.matmul(
                        ps_mm[:],
                        ma_t[:, a0 : a0 + 128].bitcast(f8),
                        mb_t[:, a0 : a0 + NW].bitcast(f8),
                        start=first,
                        stop=last,
                    )

            out_sb = enc_pool.tile([128, 2, NW], f32, tag="out")
            nc.vector.tensor_copy(out_sb[:, 0], ps_ab[:])
            nc.vector.tensor_copy(out_sb[:, 1], ps_mm[:])
            nc.sync.dma_start(out_d[0], out_sb[:, 0])
            nc.sync.dma_start(out_d[1], out_sb[:, 1])

    import bass_rust as _bass_rust

    _bass_rust.move_matmul_waits_to_ldweights(nc.m)
    _bass_rust.generate_event_semaphores(nc)
    return nc


def _get_program():
    if "nc" not in _CACHE:
        _CACHE["nc"] = _build_program()
    return _CACHE["nc"]


def _shard(x):
    x = np.asarray(x)
    if x.dtype != np.uint8:
        x = x.view(np.uint8) if x.dtype == np.bool_ else x.astype(np.uint8)
    return [
        np.ascontiguousarray(x[:, c * B_SH : (c + 1) * B_SH]).reshape(ROWS, J)
        for c in range(N_CORES)
    ]


def kernel(iris_codes_a, mask_codes_a, iris_codes_b, mask_codes_b, _trace=False):
    from concourse.bass_utils import run_bass_kernel_spmd

    nc = _get_program()
    shards = {
        "ia": _shard(iris_codes_a),
        "ma": _shard(mask_codes_a),
        "ib": _shard(iris_codes_b),
        "mb": _shard(mask_codes_b),
    }
    in_maps = [{k: v[c] for k, v in shards.items()} for c in range(N_CORES)]
    res = run_bass_kernel_spmd(nc, in_maps, list(range(N_CORES)), trace=_trace)
    _CACHE["last_result"] = res

    acc = np.zeros((2, 128, NW), np.float64)
    for r in res.results:
        acc += r["out"].astype(np.float64)

    shifts = np.arange(-R, R + 1)
    cab = np.array([np.trace(acc[0], offset=HALO + 2 * s) for s in shifts])
    den = np.array([np.trace(acc[1], offset=HALO + 2 * s) for s in shifts])
    cab = np.rint(cab * 2.0**18)
    den = np.rint(den * 2.0**18)
    num = (den - cab) / 2.0
    dist = num.astype(np.float32) / den.astype(np.float32)
    out = np.minimum(np.float32(1.0), dist.min())
    return np.asarray([out], dtype=np.float32)



# revision 2
# speedup vs baseline: 1.6150x; 1.6150x over previous
"""Masked fractional Hamming distance over 31 circular rotations, on 8 trn2 cores.

Math: for shift s, num(s)/den(s) with
  den(s) = sum_{t,k} ma_k[t] * mb_k[t+s]          (correlation at lag s)
  num(s) = masked differing bits; with the sign-encode
  A = (ia<<7)|ma, B = (ib<<7)|mb read as fp8e4m3 the bytes become
  {+0, -0, +2^-9, -2^-9} (sign=iris, magnitude=mask), so
  corr(A, B)(s) = (den - 2*num) * 2^-18, corr(A&1, B&1)(s) = den * 2^-18.

The encode happens on the HOST (halves HBM traffic: 2 tensors instead of 4);
the two k-planes are de-interleaved on the host and summed inside the PE via
fp8 DoubleRow matmuls (lhsT [K,2,128], rhs [K,2,158] -> psum [128,158] with
result = sum_i W_i.T @ X_i at 2x fp8 rate). The +-15 circular halo is baked
into B on the host. Masks (byte&1) are extracted on-device with one u16 AND.
Band diagonals are summed on the host (exact integers scaled by 2^-18).
"""

import numpy as np

N_CORES = 8
B_FULL, L = 4096, 2048
R = 15
B_SH = B_FULL // N_CORES       # 512 batches per core
ROWS = 2 * B_SH                # 1024 rows per core (2 eyes x 512 batches)
NW = 128 + 2 * R               # 158 moving window
LH = L + 2 * R                 # 2078 halo-padded plane length
N_GROUPS = ROWS // 128         # 8
N_CHUNKS = L // 128            # 16

_CACHE = {}


def _build_program():
    import concourse.bass as bass
    import concourse.tile as tile
    from concourse import mybir

    u8 = mybir.dt.uint8
    u16 = mybir.dt.uint16
    f8 = mybir.dt.float8e4
    f32 = mybir.dt.float32
    Alu = mybir.AluOpType
    DR = mybir.MatmulPerfMode.DoubleRow

    nc = bass.Bass()
    a_d = nc.declare_dram_parameter("a", [ROWS, 2, L], u8, isOutput=False)
    b_d = nc.declare_dram_parameter("b", [ROWS, 2, LH], u8, isOutput=False)
    out_d = nc.declare_dram_parameter("out", [2, 128, NW], f32, isOutput=True)

    with tile.TileContext(nc) as tc:
        with (
            tc.tile_pool(name="raw", bufs=3) as raw_pool,
            tc.tile_pool(name="acc", bufs=1, space="PSUM") as psum_pool,
        ):
            ps_ab = psum_pool.tile([128, NW], f32)
            ps_mm = psum_pool.tile([128, NW], f32)

            for g in range(N_GROUPS):
                rows = slice(g * 128, (g + 1) * 128)
                a_t = raw_pool.tile([128, 2, L], u8, tag="a")
                b_t = raw_pool.tile([128, 2, LH], u8, tag="b")
                ma_t = raw_pool.tile([128, 2, L], u8, tag="ma")
                mb_t = raw_pool.tile([128, 2, LH], u8, tag="mb")

                nc.sync.dma_start(a_t[:], a_d[rows])
                nc.sync.dma_start(b_t[:], b_d[rows])

                nc.vector.tensor_scalar(
                    ma_t[:].bitcast(u16),
                    a_t[:].bitcast(u16),
                    0x0101,
                    None,
                    op0=Alu.bitwise_and,
                )
                nc.vector.tensor_scalar(
                    mb_t[:].bitcast(u16),
                    b_t[:].bitcast(u16),
                    0x0101,
                    None,
                    op0=Alu.bitwise_and,
                )

                for c in range(N_CHUNKS):
                    a0 = c * 128
                    first = g == 0 and c == 0
                    last = g == N_GROUPS - 1 and c == N_CHUNKS - 1
                    nc.tensor.matmul(
                        ps_ab[:],
                        a_t[:, :, a0 : a0 + 128].bitcast(f8),
                        b_t[:, :, a0 : a0 + NW].bitcast(f8),
                        start=first,
                        stop=last,
                        perf_mode=DR,
                    )
                    nc.tensor.matmul(
                        ps_mm[:],
                        ma_t[:, :, a0 : a0 + 128].bitcast(f8),
                        mb_t[:, :, a0 : a0 + NW].bitcast(f8),
                        start=first,
                        stop=last,
                        perf_mode=DR,
                    )

            out_sb = raw_pool.tile([128, 2, NW], f32, tag="out")
            nc.vector.tensor_copy(out_sb[:, 0], ps_ab[:])
            nc.vector.tensor_copy(out_sb[:, 1], ps_mm[:])
            nc.sync.dma_start(out_d[0], out_sb[:, 0])
            nc.sync.dma_start(out_d[1], out_sb[:, 1])

    import bass_rust as _bass_rust

    _bass_rust.move_matmul_waits_to_ldweights(nc.m)
    _bass_rust.generate_event_semaphores(nc)
    return nc


def _get_program():
    if "nc" not in _CACHE:
        _CACHE["nc"] = _build_program()
    return _CACHE["nc"]


def _encode(iris, mask):
    """(2,B,L,2) bool pair -> (2*B, 2, L) uint8 (ia<<7)|ma, k de-interleaved."""
    enc = (iris.astype(np.uint8) << 7) | mask.astype(np.uint8)
    # (2, B, L, 2) -> (2, B, 2, L) -> (2*B, 2, L)
    return enc.transpose(0, 1, 3, 2).reshape(2 * B_FULL, 2, L)


def kernel(iris_codes_a, mask_codes_a, iris_codes_b, mask_codes_b, _trace=False):
    from concourse.bass_utils import run_bass_kernel_spmd

    nc = _get_program()

    a_full = _encode(np.asarray(iris_codes_a), np.asarray(mask_codes_a))
    b_enc = _encode(np.asarray(iris_codes_b), np.asarray(mask_codes_b))
    # circular halo of +-R on the plane axis
    b_full = np.concatenate(
        [b_enc[:, :, L - R :], b_enc, b_enc[:, :, :R]], axis=2
    )

    def rows(c):
        # rows of core c: eyes i in {0,1} x batches [c*B_SH, (c+1)*B_SH)
        idx = np.r_[c * B_SH : (c + 1) * B_SH, B_FULL + c * B_SH : B_FULL + (c + 1) * B_SH]
        return idx

    in_maps = [
        {
            "a": np.ascontiguousarray(a_full[rows(c)]),
            "b": np.ascontiguousarray(b_full[rows(c)]),
        }
        for c in range(N_CORES)
    ]
    res = run_bass_kernel_spmd(nc, in_maps, list(range(N_CORES)), trace=_trace)
    _CACHE["last_result"] = res

    acc = np.zeros((2, 128, NW), np.float64)
    for r in res.results:
        acc += r["out"].astype(np.float64)

    shifts = np.arange(-R, R + 1)
    cab = np.array([np.trace(acc[0], offset=R + s) for s in shifts])
    den = np.array([np.trace(acc[1], offset=R + s) for s in shifts])
    cab = np.rint(cab * 2.0**18)
    den = np.rint(den * 2.0**18)
    num = (den - cab) / 2.0
    dist = num.astype(np.float32) / den.astype(np.float32)
    out = np.minimum(np.float32(1.0), dist.min())
    return np.asarray([out], dtype=np.float32)


# revision 8
# speedup vs baseline: 1.6501x; 1.0217x over previous
"""Masked fractional Hamming distance over 31 circular rotations, on 8 trn2 cores.

Math: for shift s, num(s)/den(s) with
  den(s) = sum_{t,k} ma_k[t] * mb_k[t+s]          (correlation at lag s)
  num(s) = masked differing bits; with the sign-encode
  A = (ia<<7)|ma, B = (ib<<7)|mb read as fp8e4m3 the bytes become
  {+0, -0, +2^-9, -2^-9} (sign=iris, magnitude=mask), so
  corr(A, B)(s) = (den - 2*num) * 2^-18, corr(A&1, B&1)(s) = den * 2^-18.

The encode happens on the HOST (halves HBM traffic: 2 tensors instead of 4);
the two k-planes are de-interleaved on the host and summed inside the PE via
fp8 DoubleRow matmuls (lhsT [K,2,128], rhs [K,2,158] -> psum [128,158] with
result = sum_i W_i.T @ X_i at 2x fp8 rate). The +-15 circular halo is baked
into B on the host. Masks (byte&1) are extracted on-device with one u16 AND.
Each 128-row group is split into two DMA pieces (chunk 0-8 / 9-15 windows)
so compute chases the DMA stream at half-group granularity; all pieces are
prefetched (bufs=8). Band diagonals are summed on the host (exact integers
scaled by 2^-18).
"""

import numpy as np

N_CORES = 8
B_FULL, L = 4096, 2048
R = 15
B_SH = B_FULL // N_CORES       # 512 batches per core
ROWS = 2 * B_SH                # 1024 rows per core (2 eyes x 512 batches)
NW = 128 + 2 * R               # 158 moving window
LH = L + 2 * R                 # 2078 halo-padded plane length
N_GROUPS = ROWS // 128         # 8
N_CHUNKS = L // 128            # 16
C_SPLIT = 9                    # chunks 0-8 from the lo piece, 9-15 from hi
A_LO, B_LO = C_SPLIT * 128, C_SPLIT * 128 + 2 * R     # 1152, 1182
A_HI, B_HI = L - A_LO, LH - A_LO                      # 896, 926
# plane stride padded to a multiple of 16 (ldweights needs aligned strides)
W_LO = -(-(A_LO + B_LO) // 16) * 16                   # 2336
W_HI = -(-(A_HI + B_HI) // 16) * 16                   # 1824

_CACHE = {}


def _build_program():
    import concourse.bass as bass
    import concourse.tile as tile
    from concourse import mybir

    u8 = mybir.dt.uint8
    u16 = mybir.dt.uint16
    f8 = mybir.dt.float8e4
    f32 = mybir.dt.float32
    Alu = mybir.AluOpType
    DR = mybir.MatmulPerfMode.DoubleRow

    nc = bass.Bass()
    lo_d = nc.declare_dram_parameter("lo", [N_GROUPS, 128, 2, W_LO], u8, isOutput=False)
    hi_d = nc.declare_dram_parameter("hi", [N_GROUPS, 128, 2, W_HI], u8, isOutput=False)
    out_d = nc.declare_dram_parameter("out", [128, 2, NW], f32, isOutput=True)

    with tile.TileContext(nc) as tc:
        with (
            tc.tile_pool(name="raw", bufs=8) as raw_pool,
            tc.tile_pool(name="acc", bufs=1, space="PSUM") as psum_pool,
        ):
            ps_ab = psum_pool.tile([128, NW], f32)
            ps_mm = psum_pool.tile([128, NW], f32)

            for g in range(N_GROUPS):
                for half, (dram, width, a_w) in enumerate(
                    ((lo_d, W_LO, A_LO), (hi_d, W_HI, A_HI))
                ):
                    t = raw_pool.tile([128, 2, width], u8, tag=f"t{half}")
                    m = raw_pool.tile([128, 2, width], u8, tag=f"m{half}")
                    nc.sync.dma_start(t[:], dram[g])
                    nc.vector.tensor_scalar(
                        m[:].bitcast(u16),
                        t[:].bitcast(u16),
                        0x0101,
                        None,
                        op0=Alu.bitwise_and,
                    )
                    c_range = range(C_SPLIT) if half == 0 else range(C_SPLIT, N_CHUNKS)
                    for c in c_range:
                        a0 = c * 128 - (0 if half == 0 else A_LO)
                        b0 = a_w + a0
                        first = g == 0 and c == 0
                        last = g == N_GROUPS - 1 and c == N_CHUNKS - 1
                        nc.tensor.matmul(
                            ps_ab[:],
                            t[:, :, a0 : a0 + 128].bitcast(f8),
                            t[:, :, b0 : b0 + NW].bitcast(f8),
                            start=first,
                            stop=last,
                            perf_mode=DR,
                        )
                        nc.tensor.matmul(
                            ps_mm[:],
                            m[:, :, a0 : a0 + 128].bitcast(f8),
                            m[:, :, b0 : b0 + NW].bitcast(f8),
                            start=first,
                            stop=last,
                            perf_mode=DR,
                        )

            out_sb = raw_pool.tile([128, 2, NW], f32, tag="out")
            nc.vector.tensor_copy(out_sb[:, 0], ps_ab[:])
            nc.vector.tensor_copy(out_sb[:, 1], ps_mm[:])
            nc.sync.dma_start(out_d[:], out_sb[:])

    import bass_rust as _bass_rust

    _bass_rust.move_matmul_waits_to_ldweights(nc.m)
    _bass_rust.generate_event_semaphores(nc)
    return nc


def _get_program():
    if "nc" not in _CACHE:
        _CACHE["nc"] = _build_program()
    return _CACHE["nc"]


def _encode(iris, mask):
    """(2,B,L,2) bool pair -> (2*B, 2, L) uint8 (ia<<7)|ma, k de-interleaved."""
    enc = (iris.astype(np.uint8) << 7) | mask.astype(np.uint8)
    # (2, B, L, 2) -> (2, B, 2, L) -> (2*B, 2, L)
    return enc.transpose(0, 1, 3, 2).reshape(2 * B_FULL, 2, L)


def kernel(iris_codes_a, mask_codes_a, iris_codes_b, mask_codes_b, _trace=False):
    from concourse.bass_utils import run_bass_kernel_spmd

    nc = _get_program()

    a_full = _encode(np.asarray(iris_codes_a), np.asarray(mask_codes_a))
    b_enc = _encode(np.asarray(iris_codes_b), np.asarray(mask_codes_b))
    # circular halo of +-R on the plane axis
    b_full = np.concatenate(
        [b_enc[:, :, L - R :], b_enc, b_enc[:, :, :R]], axis=2
    )

    def rows(c):
        # rows of core c: eyes i in {0,1} x batches [c*B_SH, (c+1)*B_SH)
        return np.r_[
            c * B_SH : (c + 1) * B_SH, B_FULL + c * B_SH : B_FULL + (c + 1) * B_SH
        ]

    in_maps = []
    for c in range(N_CORES):
        a_c = a_full[rows(c)]
        b_c = b_full[rows(c)]
        lo = np.zeros((ROWS, 2, W_LO), np.uint8)
        hi = np.zeros((ROWS, 2, W_HI), np.uint8)
        lo[:, :, :A_LO] = a_c[:, :, :A_LO]
        lo[:, :, A_LO : A_LO + B_LO] = b_c[:, :, :B_LO]
        hi[:, :, :A_HI] = a_c[:, :, A_LO:]
        hi[:, :, A_HI : A_HI + B_HI] = b_c[:, :, A_LO:]
        in_maps.append(
            {
                "lo": lo.reshape(N_GROUPS, 128, 2, W_LO),
                "hi": hi.reshape(N_GROUPS, 128, 2, W_HI),
            }
        )
    res = run_bass_kernel_spmd(nc, in_maps, list(range(N_CORES)), trace=_trace)
    _CACHE["last_result"] = res

    acc = np.zeros((128, 2, NW), np.float64)
    for r in res.results:
        acc += r["out"].astype(np.float64)

    shifts = np.arange(-R, R + 1)
    cab = np.array([np.trace(acc[:, 0], offset=R + s) for s in shifts])
    den = np.array([np.trace(acc[:, 1], offset=R + s) for s in shifts])
    cab = np.rint(cab * 2.0**18)
    den = np.rint(den * 2.0**18)
    num = (den - cab) / 2.0
    dist = num.astype(np.float32) / den.astype(np.float32)
    out = np.minimum(np.float32(1.0), dist.min())
    return np.asarray([out], dtype=np.float32)
